# revision 22
# baseline (speedup 1.0000x reference)
"""Trainium2 Bass kernel for nn_Attention_81149112090655.

Dense ViT-style attention block with task-conditional tanh-MLP "context
prefix" terms added to k/v. Sharded data-parallel over batch B=8 across
8 NeuronCores (one batch item per core, weights replicated, no
collectives).

Layout strategy: everything is kept feature-on-partition ("transposed")
so no on-chip transposes are ever needed:
    xT [C,N] -> qkT [2C,N] -> per-head ST[j,i] -> E=exp(ST*scale)
    -> OT[d,i] = sum_j v[j,d] E[j,i]  (v computed in natural [n,c] layout
       by swapping lhsT/rhs roles on the same inputs)
    -> attnT [C,N] -> outT [C,N] = WprojT.T-style matmul + bias.
Softmax skips the max-subtraction (|S*scale| <= ~3 for these input
statistics, exp is safe in fp32/bf16); the row denominator is built by
a DVE tree-add over the 8 j-tiles plus one ones-column matmul, then
1/sum is partition-broadcast and multiplied in during PSUM evacuation.
"""

import sys

sys.path.insert(0, "/opt/trn_rl_repo")

import numpy as np
import ml_dtypes

import concourse.bass as bass
import concourse.mybir as mybir
from concourse import bacc
from concourse.tile import TileContext
from concourse.bass_utils import run_bass_kernel_spmd

BF16 = mybir.dt.bfloat16
F32 = mybir.dt.float32
NPBF16 = ml_dtypes.bfloat16

B = 8
N = 1024          # sequence length per core
D = 1152          # model dim
HEADS = 9
HD = 128          # head dim
MID = 256
MID2 = 2 * MID    # both prefix MLPs stacked
KT = D // 128     # 9 feature k-tiles
QKT = 2 * D // 128  # 18 q+k output chunks
NT = N // 128     # 8 sequence tiles
SCALE = float(HD) ** -0.5

AF = mybir.ActivationFunctionType


def _emit_accum(nc, pairs, outs):
    """Emit an accumulation group of matmuls into each (psum_ap) in outs.

    pairs: list of (lhsT_ap, rhs_full_ap) where rhs_full_ap spans the
    same free range as the outs concatenated. outs: list of
    (out_ap, rhs_slice) pairs.
    """
    n = len(pairs)
    for i, (lhsT, rhs_slices) in enumerate(pairs):
        for out_ap, rhs_ap in zip(outs, rhs_slices):
            nc.tensor.matmul(
                out_ap, lhsT=lhsT, rhs=rhs_ap, start=(i == 0), stop=(i == n - 1)
            )


def _build_nc(reps=1):
    nc = bacc.Bacc("TRN2", target_bir_lowering=False, debug=False)

    xT_d = nc.declare_dram_parameter("xT", [D, N], BF16, isOutput=False)
    wqkT_d = nc.declare_dram_parameter("wqkT", [D, 2 * D], BF16, isOutput=False)
    wvT_d = nc.declare_dram_parameter("wvT", [D, D], BF16, isOutput=False)
    uwT_d = nc.declare_dram_parameter("uwT", [MID2, 2 * D], BF16, isOutput=False)
    dwT_d = nc.declare_dram_parameter("dwT", [D, MID2], BF16, isOutput=False)
    dbT_d = nc.declare_dram_parameter("dbT", [128, MID2 // 128], F32, isOutput=False)
    qkbT_d = nc.declare_dram_parameter("qkbT", [128, QKT], F32, isOutput=False)
    vb_d = nc.declare_dram_parameter("vbias", [1, D], F32, isOutput=False)
    wpT_d = nc.declare_dram_parameter("wprojT", [D, D], BF16, isOutput=False)
    bpT_d = nc.declare_dram_parameter("bprojT", [128, KT], F32, isOutput=False)
    outT_ds = [
        nc.declare_dram_parameter("outT" if r == 0 else f"outT{r}", [D, N], F32,
                                  isOutput=True)
        for r in range(reps)
    ]

    xT_r = xT_d.rearrange("(t p) n -> t p n", p=128)
    wqkT_r = wqkT_d.rearrange("(t p) n -> t p n", p=128)
    wvT_r = wvT_d.rearrange("(t p) n -> t p n", p=128)
    uwT_r = uwT_d.rearrange("(t p) n -> t p n", p=128)
    dwT_r = dwT_d.rearrange("(t p) n -> t p n", p=128)
    wpT_r = wpT_d.rearrange("(t p) n -> t p n", p=128)

    with TileContext(nc) as tc:
      for _rep in range(reps):
        _s = f"_{_rep}" if _rep else ""
        outT_r = outT_ds[_rep].rearrange("(t p) n -> t p n", p=128)
        with (
            tc.tile_pool(name="persist" + _s, bufs=1) as persist,
            tc.tile_pool(name="wqk_s" + _s, bufs=9) as wqk_pool,
            tc.tile_pool(name="et" + _s, bufs=5) as et_pool,
            tc.tile_pool(name="padd" + _s, bufs=4) as padd_pool,
            tc.tile_pool(name="accb" + _s, bufs=1) as accb_pool,
            tc.tile_pool(name="rbc" + _s, bufs=1) as rbc_pool,
            tc.tile_pool(name="osb" + _s, bufs=2) as out_pool,
            tc.tile_pool(name="psum" + _s, bufs=2, space="PSUM") as psum_pool,
        ):
            # ---- persistent SBUF residents -------------------------------
            xT = [persist.tile([128, N], BF16, tag=f"xT{k}", name=f"xT{k}") for k in range(KT)]
            wvT = [persist.tile([128, D], BF16, tag=f"wvT{k}", name=f"wvT{k}") for k in range(KT)]
            uwT = [persist.tile([128, 2 * D], BF16, tag=f"uwT{m}", name=f"uwT{m}") for m in range(4)]
            dwT = [persist.tile([128, MID2], BF16, tag=f"dwT{k}", name=f"dwT{k}") for k in range(KT)]
            wpT = [persist.tile([128, D], BF16, tag=f"wpT{k}", name=f"wpT{k}") for k in range(KT)]
            h = [persist.tile([128, N], BF16, tag=f"h{m}", name=f"h{m}") for m in range(4)]
            qkT = [persist.tile([128, N], BF16, tag=f"qkT{m}", name=f"qkT{m}") for m in range(QKT)]
            v = [persist.tile([128, D], BF16, tag=f"v{n}", name=f"v{n}") for n in range(NT)]
            att = [persist.tile([128, N], BF16, tag=f"att{hh}", name=f"att{hh}") for hh in range(HEADS)]
            vbias = persist.tile([128, D], F32, tag="vbias", name="vbias")
            dbT = persist.tile([128, MID2 // 128], F32, tag="dbT", name="dbT")
            qkbT = persist.tile([128, QKT], F32, tag="qkbT", name="qkbT")
            bpT = persist.tile([128, KT], F32, tag="bpT", name="bpT")
            ones = persist.tile([128, 1], BF16, tag="ones", name="ones")

            # phase-B/C inputs first so PE can start ASAP, then D/F weights
            nc.sync.dma_start(out=dbT, in_=dbT_d[:, :])
            nc.sync.dma_start(out=qkbT, in_=qkbT_d[:, :])
            nc.sync.dma_start(out=bpT, in_=bpT_d[:, :])
            for k in range(KT):
                nc.sync.dma_start(out=xT[k], in_=xT_r[k])
                nc.sync.dma_start(out=dwT[k], in_=dwT_r[k])
            for m in range(4):
                nc.sync.dma_start(out=uwT[m], in_=uwT_r[m])
            nc.gpsimd.dma_start(out=vbias, in_=vb_d[:, :].to_broadcast([128, D]))
            nc.vector.memset(ones, 1.0)

            n_halves = [(0, 512), (512, 1024)]

            # ---- phase B: h = tanh(dw_comb @ xT + db) [MID2, N] ----------
            for m in range(MID2 // 128):
                ps = psum_pool.tile([128, N], F32, tag="mm", name="psmm")
                pairs = [
                    (dwT[k][:, m * 128 : (m + 1) * 128],
                     [xT[k][:, a:b] for a, b in n_halves])
                    for k in range(KT)
                ]
                _emit_accum(nc, pairs, [ps[:, a:b] for a, b in n_halves])
                nc.scalar.activation(
                    out=h[m], in_=ps, func=AF.Tanh, bias=dbT[:, m : m + 1], scale=1.0
                )

            # ---- phase C: qkT = Wqk @ xT (+ prefix for k rows) [2C, N] ---
            MB = 2  # m-chunks per weight-stream block
            for mb in range(QKT // MB):
                wt = []
                for k in range(KT):
                    t = wqk_pool.tile([128, MB * 128], BF16, tag="wqk", name="wqks")
                    nc.sync.dma_start(
                        out=t, in_=wqkT_r[k, :, mb * MB * 128 : (mb + 1) * MB * 128]
                    )
                    wt.append(t)
                for mi in range(MB):
                    m = mb * MB + mi
                    ps = psum_pool.tile([128, N], F32, tag="mm", name="psmm")
                    pairs = [
                        (wt[k][:, mi * 128 : (mi + 1) * 128],
                         [xT[k][:, a:b] for a, b in n_halves])
                        for k in range(KT)
                    ]
                    if m >= KT:  # k rows also get the prefix MLP output
                        mk = m - KT
                        pairs += [
                            (uwT[mid][:, mk * 128 : (mk + 1) * 128],
                             [h[mid][:, a:b] for a, b in n_halves])
                            for mid in range(4)
                        ]
                    _emit_accum(nc, pairs, [ps[:, a:b] for a, b in n_halves])
                    nc.scalar.activation(
                        out=qkT[m], in_=ps, func=AF.Identity,
                        bias=qkbT[:, m : m + 1], scale=1.0,
                    )

            # ---- phase D: v in natural [n, c_v] layout [N, D] ------------
            for k in range(KT):
                nc.sync.dma_start(out=wvT[k], in_=wvT_r[k])
            VS = 384  # free-dim slice (1 PSUM bank)

            def emit_vgroup(nt, sl):
                ps = psum_pool.tile([128, VS], F32, tag="mm", name="psv")
                c0, c1 = sl * VS, (sl + 1) * VS
                pairs = [
                    (xT[k][:, nt * 128 : (nt + 1) * 128], [wvT[k][:, c0:c1]])
                    for k in range(KT)
                ]
                pairs += [
                    (h[mid][:, nt * 128 : (nt + 1) * 128],
                     [uwT[mid][:, D + c0 : D + c1]])
                    for mid in range(4)
                ]
                _emit_accum(nc, pairs, [ps])
                nc.vector.tensor_add(v[nt][:, c0:c1], ps, vbias[:, c0:c1])

            # sl=2 (heads 6-8) is deferred and woven into the attention
            # stream, where ACT's exp chain leaves PE slack
            for nt in range(NT):
                for sl in range(2):
                    emit_vgroup(nt, sl)

            # ---- phase E: attention, software-pipelined ------------------
            # Flat (head, jt) slot stream with the OT (P@v) consumer skewed
            # one slot behind the ST/exp producer, so PE never sits in-order
            # behind ACT's exp. Head tails (rowsum/recip/bcast/mult) are
            # emitted right after that head's last OT.
            ot_tile = {}
            accb_tile = {}
            et_tiles = {}
            part_tiles = {}

            def head_tail(hh):
                rowsum = psum_pool.tile([1, N], F32, tag="mm", name="psrow")
                for a, b in n_halves:
                    nc.tensor.matmul(
                        rowsum[:, a:b], lhsT=ones[:, 0:1],
                        rhs=accb_tile[hh][:, a:b], start=True, stop=True,
                    )
                rbc = rbc_pool.tile([128, N], F32, tag="rbc", name="rbc")
                nc.vector.reciprocal(rbc[0:1, :], rowsum)
                nc.gpsimd.partition_broadcast(rbc, rbc[0:1, :])
                nc.vector.tensor_mul(att[hh], ot_tile[hh], rbc)

            def emit_ot(hh, jt):
                if jt == 0:
                    ot_tile[hh] = psum_pool.tile(
                        [128, N], F32, tag="ot", name="psot"
                    )
                vslice = v[jt][:, hh * 128 : (hh + 1) * 128]
                et = et_tiles.pop((hh, jt))
                for a, b in n_halves:
                    nc.tensor.matmul(
                        ot_tile[hh][:, a:b], lhsT=vslice, rhs=et[:, a:b],
                        start=(jt == 0), stop=(jt == NT - 1),
                    )
                if jt == NT - 1:
                    pending_tails.append(hh)

            slots = [(hh, jt) for hh in range(HEADS) for jt in range(NT)]
            pending_tails = []
            for idx, (hh, jt) in enumerate(slots):
                qT_h = qkT[hh]
                kT_h = qkT[KT + hh]
                # deferred head tail: emit once accb has been ready a while,
                # so the rowsum psum slot is held only briefly
                if jt == 4 and pending_tails:
                    head_tail(pending_tails.pop(0))
                if idx % 6 == 3 and idx // 6 < NT:
                    emit_vgroup(idx // 6, 2)
                st = psum_pool.tile([128, N], F32, tag="mm", name="psmm")
                kslice = kT_h[:, jt * 128 : (jt + 1) * 128]
                for a, b in n_halves:
                    nc.tensor.matmul(
                        st[:, a:b], lhsT=kslice, rhs=qT_h[:, a:b],
                        start=True, stop=True,
                    )
                et = et_pool.tile([128, N], BF16, tag="et", name="et")
                nc.scalar.activation(out=et, in_=st, func=AF.Exp, scale=SCALE)
                et_tiles[(hh, jt)] = et
                # pairwise bf16 partial-sum tree over the 8 j-tiles (the
                # cross-partition sum happens later via the ones-matmul)
                if jt % 2 == 1:
                    p = padd_pool.tile([128, N], BF16, tag="padd", name="padd")
                    nc.vector.tensor_add(p, et_tiles[(hh, jt - 1)], et)
                    part_tiles[(hh, 0, jt // 2)] = p
                if jt == 3 or jt == NT - 1:
                    q = padd_pool.tile([128, N], BF16, tag="padd", name="padd")
                    j2 = jt // 2
                    nc.vector.tensor_add(
                        q, part_tiles.pop((hh, 0, j2 - 1)),
                        part_tiles.pop((hh, 0, j2)),
                    )
                    part_tiles[(hh, 1, jt // 4)] = q
                if jt == NT - 1:
                    accb = accb_pool.tile([128, N], BF16, tag="accb", name="accb")
                    nc.vector.tensor_add(
                        accb, part_tiles.pop((hh, 1, 0)), part_tiles.pop((hh, 1, 1))
                    )
                    accb_tile[hh] = accb
                if idx > 1:
                    emit_ot(*slots[idx - 2])
            emit_ot(*slots[-2])
            emit_ot(*slots[-1])
            while pending_tails:
                head_tail(pending_tails.pop(0))

            # ---- phase F: outT = Wproj @ attnT + bproj [D, N] ------------
            for k in range(KT):
                nc.sync.dma_start(out=wpT[k], in_=wpT_r[k])
            for m in range(KT):
                ps = psum_pool.tile([128, N], F32, tag="mm", name="psmm")
                pairs = [
                    (wpT[k][:, m * 128 : (m + 1) * 128],
                     [att[k][:, a:b] for a, b in n_halves])
                    for k in range(KT)
                ]
                _emit_accum(nc, pairs, [ps[:, a:b] for a, b in n_halves])
                osb = out_pool.tile([128, N], F32, tag="osb", name="osb")
                nc.scalar.activation(
                    out=osb, in_=ps, func=AF.Identity,
                    bias=bpT[:, m : m + 1], scale=1.0,
                )
                nc.sync.dma_start(out=outT_r[m], in_=osb)

    nc.compile()
    return nc


_NC_CACHE = None


def _get_nc():
    global _NC_CACHE
    if _NC_CACHE is None:
        _NC_CACHE = _build_nc()
    return _NC_CACHE


def _bf16(a):
    return np.ascontiguousarray(a).astype(NPBF16)


def _host_prep(inputs):
    x = np.asarray(inputs["x"], dtype=np.float32)
    Wqkv = np.asarray(inputs["Wqkv"], dtype=np.float32)
    Wproj = np.asarray(inputs["Wproj"], dtype=np.float32)
    bproj = np.asarray(inputs["bproj"], dtype=np.float32)
    task = int(np.asarray(inputs["task"]))

    def _get(name, shape, active):
        if active:
            return np.asarray(inputs[name], dtype=np.float32)
        return np.zeros(shape, dtype=np.float32)

    use1 = task >= 4  # down/up MLP
    use2 = task >= 5  # pdown/pup MLP
    dw1 = _get("down_w", (MID, D), use1)
    db1 = _get("down_b", (MID,), use1)
    uw1 = _get("up_w", (2 * D, MID), use1)
    ub1 = _get("up_b", (2 * D,), use1)
    dw2 = _get("pdown_w", (MID, D), use2)
    db2 = _get("pdown_b", (MID,), use2)
    uw2 = _get("pup_w", (2 * D, MID), use2)
    ub2 = _get("pup_b", (2 * D,), use2)

    dw_comb = np.concatenate([dw1, dw2], axis=0)      # [512, 1152]
    db_comb = np.concatenate([db1, db2], axis=0)      # [512]
    uw_comb = np.concatenate([uw1, uw2], axis=1)      # [2304, 512]
    ub_comb = ub1 + ub2                               # [2304]

    WqkvT = Wqkv.T  # [1152, 3456]
    common = {
        "wqkT": _bf16(WqkvT[:, : 2 * D]),
        "wvT": _bf16(WqkvT[:, 2 * D :]),
        "uwT": _bf16(uw_comb.T),
        "dwT": _bf16(dw_comb.T),
        "dbT": np.ascontiguousarray(db_comb.reshape(MID2 // 128, 128).T),
        "qkbT": np.ascontiguousarray(
            np.concatenate([np.zeros(D, np.float32), ub_comb[:D]])
            .reshape(QKT, 128).T
        ),
        "vbias": np.ascontiguousarray(ub_comb[D:].reshape(1, D)),
        "wprojT": _bf16(Wproj.T),
        "bprojT": np.ascontiguousarray(bproj.reshape(KT, 128).T),
    }
    in_maps = [dict(common, xT=_bf16(x[b].T)) for b in range(B)]
    return in_maps, task


def _assemble_out(results):
    out = np.stack([results[b]["outT"].T for b in range(B)], axis=0)
    return np.ascontiguousarray(out, dtype=np.float32)


def kernel(**inputs):
    in_maps, task = _host_prep(inputs)
    nc = _get_nc()
    res = run_bass_kernel_spmd(nc, in_maps, core_ids=list(range(B)))
    out = _assemble_out(res.results)
    task_prefix = np.asarray(inputs["down_w"], np.float32) if task >= 4 else None
    return (out, task_prefix)


if __name__ == "__main__":
    rng = np.random.default_rng(0)
    s = 0.02
    demo = {
        "x": rng.standard_normal((B, N, D), dtype=np.float32),
        "Wqkv": rng.standard_normal((3 * D, D), dtype=np.float32) * s,
        "Wproj": rng.standard_normal((D, D), dtype=np.float32) * s,
        "bproj": rng.standard_normal((D,), dtype=np.float32) * s,
        "down_w": rng.standard_normal((MID, D), dtype=np.float32) * s,
        "down_b": rng.standard_normal((MID,), dtype=np.float32) * s,
        "up_w": rng.standard_normal((2 * D, MID), dtype=np.float32) * s,
        "up_b": rng.standard_normal((2 * D,), dtype=np.float32) * s,
        "pdown_w": rng.standard_normal((MID, D), dtype=np.float32) * s,
        "pdown_b": rng.standard_normal((MID,), dtype=np.float32) * s,
        "pup_w": rng.standard_normal((2 * D, MID), dtype=np.float32) * s,
        "pup_b": rng.standard_normal((2 * D,), dtype=np.float32) * s,
        "task": 5,
    }
    out, tp = kernel(**demo)
    print("out", out.shape, out.dtype, "prefix", None if tp is None else tp.shape)
